# revision 6
# baseline (speedup 1.0000x reference)
"""Trainium2 Bass kernel for nn_BioRNN_working_42142219109075.

Strategy (tensor-parallel over the neuron axis, 8 cores):
  - N=3840 neurons padded to 4096, permuted so every core owns a uniform
    512-column shard: [224 soma | 256 dendrite | 32 pad].
  - Dendrite->soma coupling and the DECAY factor are folded into the
    recurrent weight matrix host-side, so the per-step update is
        h' = 0.8*h + r @ Ws[:, shard] + ext_t,   r = relu/tanh(h')
    with ext_t = 0.2*(x_t @ w_in + b) + noise_scale*noise_t precomputed.
  - Each core keeps r^T (full, [4096,128]) in SBUF for the matmul
    (stationary lhsT = r^T k-tile, moving rhs = W k-tile [128,512]).
  - Per step: 32 accumulating matmuls -> PSUM [128,512]; DVE h-update;
    ACT relu/tanh; 4 PE transposes of the new r shard; AllGather of the
    transposed shard across the 8 cores; reload r^T tiles; readout
    matmuls (SR e-soma k-tiles only) -> out[t].
  - Matmul dtype float32r: full PE rate at free-dim 512, ~1.5e-4 matmul
    relative error (vs 2.5e-3 for bf16).
"""
import numpy as np

import concourse.bass as bass
import concourse.bacc as bacc
import concourse.mybir as mybir
import concourse.tile as tile
from concourse import masks
from concourse.bass_utils import run_bass_kernel_spmd

# ---- problem constants (hardcoded; kernel.py must be self-contained) ----
T, B, N_IN, N_OUT = 200, 128, 64, 10
N = 3840
NP = 4096          # padded neuron count
CORES = 8
SH = NP // CORES   # 512 columns per core
KT = NP // 128     # 32 k-tiles
NSOMA = 224        # soma columns per shard (relu); rest tanh
DECAY = 0.2
NOISE_SCALE = float(np.sqrt(2.0 * DECAY) * 0.01)
# k-tiles holding SR e-soma rows in the permuted layout (readout support)
RO_KTILES = (0, 1, 4, 5, 8)

DT_NAME = "f32r"   # "f32r" | "bf16" | "f32"

_DT_MAP = {
    "f32r": mybir.dt.float32r,
    "bf16": mybir.dt.bfloat16,
    "f32": mybir.dt.float32,
}
F32 = mybir.dt.float32


def _build_perm():
    soma = np.concatenate([
        np.arange(0, 512), np.arange(1536, 1920),
        np.arange(1920, 2432), np.arange(3456, 3840)])
    dend = np.concatenate([np.arange(512, 1536), np.arange(2432, 3456)])
    perm = np.full(NP, -1, dtype=np.int64)
    for c in range(CORES):
        base = c * SH
        perm[base:base + NSOMA] = soma[c * NSOMA:(c + 1) * NSOMA]
        perm[base + NSOMA:base + NSOMA + 256] = dend[c * 256:(c + 1) * 256]
    return perm


def build_nc(t_steps=T, dt_name=DT_NAME):
    dt = _DT_MAP[dt_name]
    r_dt = mybir.dt.bfloat16 if dt_name == "bf16" else F32

    nc = bacc.Bacc("TRN2", target_bir_lowering=False, debug=False,
                   num_devices=CORES)
    w_in_d = nc.dram_tensor("w", [NP, SH], dt, kind="ExternalInput").ap()
    wo_d = nc.dram_tensor("wo", [len(RO_KTILES) * 128, N_OUT], dt,
                          kind="ExternalInput").ap()
    rt0_d = nc.dram_tensor("rt0", [NP, B], dt, kind="ExternalInput").ap()
    h0_d = nc.dram_tensor("h0", [B, SH], F32, kind="ExternalInput").ap()
    ext_d = nc.dram_tensor("ext", [t_steps, B, SH], F32,
                           kind="ExternalInput").ap()
    out_d = nc.dram_tensor("out", [t_steps, B, N_OUT], F32,
                           kind="ExternalOutput").ap()

    nro = len(RO_KTILES)
    with tile.TileContext(nc) as tc:
        with (
            tc.tile_pool(name="const", bufs=1) as const_pool,
            tc.tile_pool(name="rt", bufs=2) as rt_pool,
            tc.tile_pool(name="state", bufs=2) as state_pool,
            tc.tile_pool(name="work", bufs=3) as work_pool,
            tc.tile_pool(name="psmain", bufs=2, space="PSUM") as ps_main,
            tc.tile_pool(name="pstr", bufs=2, space="PSUM") as ps_tr,
            tc.tile_pool(name="psro", bufs=2, space="PSUM") as ps_ro,
            tc.tile_pool(name="dram", bufs=2, space="DRAM") as dram_pool,
        ):
            # ---- constants ----
            w_sb = [const_pool.tile([128, SH], dt, tag=f"w{k}", name=f"w{k}")
                    for k in range(KT)]
            wv = w_in_d.rearrange("(k p) n -> k p n", p=128)
            for k in range(KT):
                nc.sync.dma_start(w_sb[k][:, :], wv[k])
            wo_sb = const_pool.tile([128, nro * N_OUT], dt)
            wov = wo_d.rearrange("(j p) o -> j p o", p=128)
            for j in range(nro):
                nc.sync.dma_start(wo_sb[:, j * N_OUT:(j + 1) * N_OUT], wov[j])
            ident = const_pool.tile([128, 128], r_dt)
            masks.make_identity(nc, ident[:, :])

            # ---- initial state ----
            rT = [rt_pool.tile([128, B], dt, tag=f"rt{k}", name=f"rt{k}")
                  for k in range(KT)]
            rt0v = rt0_d.rearrange("(k p) b -> k p b", p=128)
            for k in range(KT):
                nc.sync.dma_start(rT[k][:, :], rt0v[k])
            h_cur = state_pool.tile([B, SH], F32, tag="h")
            nc.sync.dma_start(h_cur[:, :], h0_d)

            for t in range(t_steps):
                # ext prefetch
                ext_t = work_pool.tile([B, SH], F32, tag="ext")
                nc.sync.dma_start(ext_t[:, :], ext_d[t])

                # main matmul: psum[b, n_shard] += rT_k.T @ W_k
                ps = ps_main.tile([B, SH], F32, tag="ps")
                for k in range(KT):
                    nc.tensor.matmul(ps[:, :], rT[k][:, :], w_sb[k][:, :],
                                     start=(k == 0), stop=(k == KT - 1))

                # h update: h_new = 0.8*h + psum + ext
                t1 = work_pool.tile([B, SH], F32, tag="t1")
                nc.vector.scalar_tensor_tensor(
                    t1[:, :], h_cur[:, :], 0.8, ps[:, :],
                    mybir.AluOpType.mult, mybir.AluOpType.add)
                h_new = state_pool.tile([B, SH], F32, tag="h")
                nc.vector.tensor_tensor(h_new[:, :], t1[:, :], ext_t[:, :],
                                        mybir.AluOpType.add)

                # activation (soma relu / dendrite+pad tanh)
                r_t = work_pool.tile([B, SH], r_dt, tag="r")
                nc.scalar.activation(r_t[:, :NSOMA], h_new[:, :NSOMA],
                                     mybir.ActivationFunctionType.Relu)
                nc.scalar.activation(r_t[:, NSOMA:], h_new[:, NSOMA:],
                                     mybir.ActivationFunctionType.Tanh)

                # transpose own shard: 4 blocks of [128,128]
                ptr = ps_tr.tile([128, SH], F32, tag="ptr")
                for j in range(4):
                    nc.tensor.transpose(ptr[:, j * 128:(j + 1) * 128],
                                        r_t[:, j * 128:(j + 1) * 128],
                                        ident[:, :])
                tmp_rt = work_pool.tile([128, SH], dt, tag="tmp_rt")
                nc.vector.tensor_copy(tmp_rt[:, :], ptr[:, :])

                # ship to DRAM, AllGather, reload full r^T
                ag_in = dram_pool.tile([SH, B], dt, tag="ag_in")
                for j in range(4):
                    nc.sync.dma_start(ag_in[j * 128:(j + 1) * 128, :],
                                      tmp_rt[:, j * 128:(j + 1) * 128])
                ag_out = dram_pool.tile([NP, B], dt, tag="ag_out")
                nc.gpsimd.collective_compute(
                    "AllGather", mybir.AluOpType.bypass,
                    replica_groups=[list(range(CORES))],
                    ins=[ag_in.opt()], outs=[ag_out.opt()])
                agv = ag_out[:, :].rearrange("(k p) b -> k p b", p=128)
                rT = [rt_pool.tile([128, B], dt, tag=f"rt{k}", name=f"rtn{k}")
                      for k in range(KT)]
                for k in range(KT):
                    nc.sync.dma_start(rT[k][:, :], agv[k])

                # readout of r(t) from gathered r^T -> out[t]
                pro = ps_ro.tile([B, N_OUT], F32, tag="pro")
                for j, kt in enumerate(RO_KTILES):
                    nc.tensor.matmul(pro[:, :], rT[kt][:, :],
                                     wo_sb[:, j * N_OUT:(j + 1) * N_OUT],
                                     start=(j == 0), stop=(j == nro - 1))
                ro_sb = work_pool.tile([B, N_OUT], F32, tag="ro")
                nc.scalar.copy(ro_sb[:, :], pro[:, :])
                nc.sync.dma_start(out_d[t], ro_sb[:, :])

                h_cur = h_new

    nc.compile()
    return nc


def host_precompute(x, noise, w_rec, w_in, w_out, b, h0, mask, is_dend):
    x = np.ascontiguousarray(np.asarray(x, np.float32))
    noise = np.asarray(noise, np.float32)
    w_rec = np.asarray(w_rec, np.float32)
    w_in = np.asarray(w_in, np.float32)
    w_out = np.asarray(w_out, np.float32)
    b = np.asarray(b, np.float32)
    h0 = np.asarray(h0, np.float32)
    mask = np.asarray(mask, np.float32)
    is_dend = np.asarray(is_dend)
    t_steps = x.shape[0]

    w_eff = np.abs(w_rec) * mask
    # fold dendrite->soma coupling: total[:, es+n] += r[:, ed0+n] + r[:, ed1+n]
    for es0, ed0 in ((0, 512), (1920, 2432)):
        for br in range(2):
            rows = np.arange(ed0 + br * 512, ed0 + (br + 1) * 512)
            w_eff[rows, np.arange(es0, es0 + 512)] += 1.0
    ws = DECAY * w_eff

    ext = DECAY * (x @ w_in + b[None, None, :]) + NOISE_SCALE * noise

    perm = _build_perm()
    valid = perm >= 0
    pv = perm[valid]

    wpp = np.zeros((NP, NP), np.float32)
    wpp[np.ix_(valid, valid)] = ws[np.ix_(pv, pv)]
    extp = np.zeros((t_steps, B, NP), np.float32)
    extp[:, :, valid] = ext[:, :, pv]
    h0p = np.zeros((B, NP), np.float32)
    h0p[:, valid] = h0[:, pv]
    r0 = np.where(is_dend[None, :], np.tanh(h0), np.maximum(h0, 0.0))
    rt0 = np.zeros((NP, B), np.float32)
    rt0[valid, :] = r0[:, pv].T

    w_out_p = np.zeros((NP, N_OUT), np.float32)
    es_pos = valid.copy()
    es_pos[valid] = pv < 512
    w_out_p[es_pos] = w_out[perm[es_pos]]
    wo_packed = np.concatenate(
        [w_out_p[kt * 128:(kt + 1) * 128] for kt in RO_KTILES], axis=0)
    return wpp, extp, h0p, rt0, wo_packed


_NC_CACHE = {}
LAST_RESULT = None


def kernel(**inputs):
    wpp, extp, h0p, rt0, wo_packed = host_precompute(**inputs)
    t_steps = extp.shape[0]

    key = (t_steps, DT_NAME)
    if key not in _NC_CACHE:
        _NC_CACHE[key] = build_nc(t_steps, DT_NAME)
    nc = _NC_CACHE[key]

    if DT_NAME == "bf16":
        import ml_dtypes

        def cast(a):
            return np.asarray(a, ml_dtypes.bfloat16)
    else:
        def cast(a):
            return a

    in_maps = []
    for c in range(CORES):
        in_maps.append({
            "w": cast(np.ascontiguousarray(wpp[:, c * SH:(c + 1) * SH])),
            "wo": cast(wo_packed),
            "rt0": cast(rt0),
            "h0": np.ascontiguousarray(h0p[:, c * SH:(c + 1) * SH]),
            "ext": np.ascontiguousarray(extp[:, :, c * SH:(c + 1) * SH]),
        })
    import os
    trace = bool(int(os.environ.get("BIORNN_TRACE", "0")))
    res = run_bass_kernel_spmd(nc, in_maps, list(range(CORES)), trace=trace)
    global LAST_RESULT
    LAST_RESULT = res
    return np.asarray(res.results[0]["out"], np.float32)
